# revision 29
# baseline (speedup 1.0000x reference)
"""Trainium2 Bass kernel for the parameterized-quantum-circuit policy network.

Math: the circuit is psi = V5 E4 V4 ... E0 V0 e0 where V_l are x-independent
1024x1024 unitaries (single-qubit rotations + CZ ring, all built from theta)
and E_l(x) = tensor-prod of Rx(lam*x). Using Rx = H Rz H (H = Hadamard^{ox10}),
E_l = H D_l(x) H with D_l diagonal. Folding the H's into the V's:

    psi = W5 D4 W4 D3 W3 D2 W2 D1 W1 (D0 * psi1)

with W_l = H V_l H (l=1..4), W5 = V5 H, psi1 = first column of H V0, and
D_l[b,k] = exp(-i * phi), phi = sum_q (1-2 bits[k,q]) * lam[l,q] * x[b,q] / 2.

Device work per core (batch-sharded 2048 -> 8 x 256, state [1024, 256] with
dim on partitions, all fp16 except phases in f32 PSUM): per layer one complex
1024x1024 matmul via Karatsuba (3 fp16 matmuls k1=C@a, k2=D@b, k3=(C+D)@(a+b);
re=k1-k2, im=k3-k1-k2), fused with the diagonal phase multiply during PSUM
evacuation. Phases: one K=12 matmul per k-tile emits [phi | phi+0.25] into a
single PSUM bank (doubled-column xt with a [zeros|ones] row gates the +0.25,
which doubles as the pi/2 cosine bias; psi1's phase rides row 10 for l=0);
rint range reduction via the 1.5*2^23 magic constant with the rounding on ACT
(Identity, scale=-1, bias=-MAGIC) and one DVE op recovering the fraction;
both sin and cos come from one wide ACT Sin. Pipelining: mi-halves of a pass
share one start-zeroed PSUM bank per Karatsuba term (3 banks/pass,
double-buffered), next-layer (and next-round l=0/1) phase matmuls fill the
PE pass gaps, ACT stages k1/k3 to SBUF so banks free early, and gpsimd is
kept to two SBUF-only ops per pass (real-HW gpsimd runs far below the cost
model's rate). Readout sum(|psi|^2 * Zsign) via M=1 reduce matmuls, sigmoid
for the 2-way softmax. All theta/lam/w-derived tables are host-precomputed;
all x-dependent compute runs on device.
"""

import sys

sys.path.insert(0, "/opt/trn_rl_repo")

import numpy as np
import concourse.bass as bass
import concourse.mybir as mybir
import concourse.tile as tile
from concourse.bass_utils import run_bass_kernel_spmd

F32 = mybir.dt.float32
F16 = mybir.dt.float16
AF = mybir.ActivationFunctionType
ALU = mybir.AluOpType

NQ = 10
DIM = 1024
L = 5
B = 2048
NC = 8
BC = B // NC  # 256 batch per core
KT = DIM // 128  # 8 k tiles
BETA = 1.0

PI = float(np.pi)
MAGIC = float(1.5 * 2**23)
TWOPI = float(2.0 * np.pi)


# ---------------------------------------------------------------- host math
_bits = (np.arange(DIM)[:, None] >> (NQ - 1 - np.arange(NQ))) & 1
_SIGNS = (1.0 - 2.0 * _bits).astype(np.float64)
_cz = np.ones(DIM)
for _i in range(NQ):
    _cz *= 1.0 - 2.0 * (_bits[:, _i] * _bits[:, (_i + 1) % NQ])
_ZSIGN = (1.0 - 2.0 * (_bits.sum(1) % 2)).astype(np.float64)


def _rx(t):
    c, s = np.cos(0.5 * t), np.sin(0.5 * t)
    return np.array([[c, -1j * s], [-1j * s, c]])


def _ry(t):
    c, s = np.cos(0.5 * t), np.sin(0.5 * t)
    return np.array([[c, -s], [s, c]])


def _rz(t):
    e = np.exp(-0.5j * t)
    return np.array([[e, 0.0], [0.0, np.conj(e)]])


def _build_weights(theta, lam):
    th = np.asarray(theta, np.float64).reshape(L + 1, NQ, 3)
    lm = np.asarray(lam, np.float64).reshape(L, NQ)
    H1 = np.array([[1.0, 1.0], [1.0, -1.0]]) / np.sqrt(2.0)
    H = np.array([[1.0]])
    for _ in range(NQ):
        H = np.kron(H, H1)
    V = []
    for l in range(L + 1):
        U = np.array([[1.0]], dtype=np.complex128)
        for q in range(NQ):
            U = np.kron(U, _rz(th[l, q, 2]) @ _ry(th[l, q, 1]) @ _rx(th[l, q, 0]))
        V.append(_cz[:, None] * U)
    psi1 = (H @ V[0])[:, 0]
    W = [H @ V[l] @ H for l in range(1, L)] + [V[L] @ H]
    A = np.empty((L, NQ, DIM))
    for l in range(L):
        A[l] = (_SIGNS * (lm[l] / 2.0)).T
    return W, psi1, A


# ---------------------------------------------------------------- device IR
def _legalize_single_wait(nc):
    """This walrus build accepts only one sync-wait per instruction: hoist
    extra waits onto injected single-wait EventSemaphore carriers."""
    n_fix = 0
    for f in nc.m.functions:
        for bb in f.blocks:
            insts = bb.instructions
            new = []
            for ins in insts:
                si = ins.sync_info
                if si is not None and len(si.on_wait) > 1:
                    for w in si.on_wait[:-1]:
                        n_fix += 1
                        ev = mybir.InstEventSemaphore(
                            name=f"waitfix_{ins.name}_{n_fix}", ins=[], outs=[]
                        )
                        ev.engine = ins.engine
                        ev.sync_info = mybir.SyncInfo(on_wait=[w], on_update=[])
                        new.append(ev)
                    ins.sync_info = mybir.SyncInfo(
                        on_wait=[si.on_wait[-1]], on_update=si.on_update
                    )
                new.append(ins)
            insts[:] = new
    return n_fix


def _build_nc(mm_f32r=False, debug=False, repeat=1, internal_weights=False):
    nc = bass.Bass()
    wkind = "Internal" if internal_weights else "ExternalInput"

    # xt rows 0..9 = [x.T | x.T]; row 10 = ones; row 11 = [zeros | ones]
    # (row 11 activates the +0.25 cosine branch only in the second half)
    xt_d = nc.dram_tensor("xt", [NQ + 2, 2 * BC], F16, kind="ExternalInput")
    # at rows 0..9 = A/2pi, row 10 = -angle(psi1)/2pi (l=0 only; folds the
    # initial-state phase into the l=0 phase tables), row 11 = 0.25
    at_d = nc.dram_tensor("at", [NQ + 2, L, DIM], F16, kind="ExternalInput")
    psire_d = nc.dram_tensor("psire", [128, KT], F32, kind="ExternalInput")
    psiim_d = nc.dram_tensor("psiim", [128, KT], F32, kind="ExternalInput")
    zs_d = nc.dram_tensor("zs", [128, KT], F16, kind="ExternalInput")
    wsc_d = nc.dram_tensor("wsc", [1, 1], F32, kind="ExternalInput")
    wall_d = {}
    for l in range(1, L + 1):
        # [pass, 128, k-tile, P1mi0|P1mi1|P2mi0|P2mi1|P3mi0|P3mi1]
        wall_d[l] = nc.dram_tensor(f"wall{l}", [4, 128, KT, 768], F16, kind=wkind)
    probs_d = nc.dram_tensor("probs", [2, BC], F32, kind="ExternalOutput")
    if debug:
        dbga_d = nc.dram_tensor("dbga", [L + 1, 128, KT, BC], F16, kind="ExternalOutput")
        dbgb_d = nc.dram_tensor("dbgb", [L + 1, 128, KT, BC], F16, kind="ExternalOutput")

    with tile.TileContext(nc) as tc:
        with (
            tc.tile_pool(name="consts", bufs=1) as cpool,
            tc.tile_pool(name="state", bufs=2) as spool,
            tc.tile_pool(name="wts", bufs=4) as wpool,
            tc.tile_pool(name="trig", bufs=2) as tpool,
            tc.tile_pool(name="scr", bufs=6) as upool,
            tc.tile_pool(name="outp", bufs=1) as opool,
            tc.tile_pool(name="psum", bufs=1, space="PSUM") as ppool,
        ):
            # ---- constants
            xt_t = cpool.tile([NQ + 2, 2 * BC], F16)
            nc.sync.dma_start(xt_t[:], xt_d[:])
            at_t = cpool.tile([NQ + 2, L, DIM], F16)
            nc.sync.dma_start(at_t[:], at_d[:])
            psire_t = cpool.tile([128, KT], F32)
            nc.sync.dma_start(psire_t[:], psire_d[:])
            psiim_t = cpool.tile([128, KT], F32)
            nc.sync.dma_start(psiim_t[:], psiim_d[:])
            zs_t = cpool.tile([128, KT], F16)
            nc.sync.dma_start(zs_t[:], zs_d[:])
            wsc_t = cpool.tile([1, 1], F32)
            nc.sync.dma_start(wsc_t[:], wsc_d[:])
            zbias = cpool.tile([128, 1], F32)
            nc.vector.memset(zbias[:], 0.0)
            nmag_b = cpool.tile([128, 1], F32)
            nc.vector.memset(nmag_b[:], -MAGIC)
            zb1 = cpool.tile([1, 1], F32)
            nc.vector.memset(zb1[:], 0.0)

            def phase_tile(l, cs_t, t):
                """cs_t[:, t] [128, 2, BC] fp16 <- sin (slot 0) / cos (slot
                1) of 2pi*phi' for k-tile t of layer l. phi' = phi/2pi from
                the PE (A tables pre-divided by 2pi); phi'+0.25 from the
                11th ones-row. One MAGIC rint range reduction covers both:
                sin(2pi*(phi'+0.25 - rint(phi'+0.25))) == cos(2pi*phi'), so
                the cosine slot needs no bias and both slots share the wide
                ops. One 2KB PSUM bank holds phi (region 0) and phi+0.25
                (region 1) as a single accumulation group: start zeroes the
                bank, the second matmul lands on zeroes."""
                ph = ppool.tile([128, 2, BC], F32, tag="ph", name="ph", bufs=2)
                nc.tensor.matmul(
                    ph[:],
                    at_t[:, l, 128 * t : 128 * (t + 1)],
                    xt_t[:],
                    start=True,
                    stop=True,
                    skip_group_check=True,
                )
                # ACT does the rint: n' = -(MAGIC + rint(ph)); then one
                # DVE op recovers the fraction fr = (n' + MAGIC) + ph
                n12 = upool.tile([128, 2, BC], F32, tag="rn1", name="n12", bufs=2)
                nc.scalar.activation(
                    n12[:], ph[:], AF.Identity, bias=nmag_b[:], scale=-1.0
                )
                fr12 = upool.tile([128, 2, BC], F32, tag="rf1", name="fr12", bufs=2)
                nc.vector.scalar_tensor_tensor(
                    fr12[:], n12[:], MAGIC, ph[:], ALU.add, ALU.add
                )
                nc.scalar.activation(
                    cs_t[:, t, :, :], fr12[:], AF.Sin, bias=zbias[:], scale=TWOPI
                )

            def emit_round(dump_debug, cs_pre):
                # ---- init: state = D_0 * psi1  (a+ib, fp16)
                if cs_pre is None:
                    cs = {0: tpool.tile([128, KT, 2, BC], F16, tag="cs", name="cs0")}
                    for t in range(KT):
                        phase_tile(0, cs[0], t)
                else:
                    cs = dict(cs_pre)
                a_t = spool.tile([128, KT, BC], F16, tag="sa", name="a0")
                b_t = spool.tile([128, KT, BC], F16, tag="sb", name="b0")
                s_t = spool.tile([128, KT, BC], F16, tag="ss", name="s0")
                for t in range(KT):
                    # psi1's phase is folded into the l=0 tables, so
                    # state0 = |psi1| * (cos - i sin)
                    nc.vector.tensor_scalar_mul(
                        a_t[:, t, :], cs[0][:, t, 1, :], psire_t[:, t : t + 1]
                    )
                    nc.vector.tensor_scalar_mul(
                        b_t[:, t, :], cs[0][:, t, 0, :], psiim_t[:, t : t + 1]
                    )
                nc.vector.tensor_add(s_t[:], a_t[:], b_t[:])
                if dump_debug:
                    nc.sync.dma_start(dbga_d[0], a_t[:])
                    nc.sync.dma_start(dbgb_d[0], b_t[:])
                if cs_pre is None:
                    cs[1] = tpool.tile([128, KT, 2, BC], F16, tag="cs", name="cs1")
                    for t in range(KT):
                        phase_tile(1, cs[1], t)
                csn = {}

                # ---- layers
                for l in range(1, L + 1):
                    if l < L:
                        csl = cs[l]
                        a2_t = spool.tile([128, KT, BC], F16, tag="sa", name="a2")
                        b2_t = spool.tile([128, KT, BC], F16, tag="sb", name="b2")
                        s2_t = spool.tile([128, KT, BC], F16, tag="ss", name="s2")
                    else:
                        sq_t = spool.tile([128, KT, BC], F16, tag="sa", name="sq")
                    if l + 1 < L:
                        # phases for layer l+1, computed in layer-l pass gaps
                        fill_l = l + 1
                        fill_cs = cs[l + 1] = tpool.tile(
                            [128, KT, 2, BC], F16, tag="cs", name=f"cs{l + 1}"
                        )
                    else:
                        # layers 4/5 gaps prefetch next round's l=0/1 phases
                        fill_l = 0 if l == L - 1 else 1
                        fill_cs = csn[fill_l] = tpool.tile(
                            [128, KT, 2, BC], F16, tag="cs", name=f"csn{fill_l}"
                        )
                    dmas = (nc.sync, nc.scalar)
                    for pass_ in range(4):
                        # one big weight DMA per pass: dram [128, k, 768]
                        # -> SBUF [128, k, 768] (few DGE issues, same bytes)
                        wt = wpool.tile(
                            [128, KT, 768], F16, tag="wall", name="wallt", bufs=6
                        )
                        dmas[pass_ % 2].dma_start(wt[:], wall_d[l][pass_])
                        # k1/k2/k3 hold both mi halves of one pass in a
                        # single bank each (one accumulation group; the
                        # second half lands on start-zeroed PSUM); bufs=2
                        # double-buffers consecutive passes
                        k1t = ppool.tile([128, 2, BC], F32, tag="k1", name="k1", bufs=2)
                        k2t = ppool.tile([128, 2, BC], F32, tag="k2", name="k2", bufs=2)
                        k3t = ppool.tile([128, 2, BC], F32, tag="k3", name="k3", bufs=2)
                        for k in range(KT):
                            for mi in range(2):
                                p1 = wt[:, k, 128 * mi : 128 * mi + 128]
                                p2 = wt[:, k, 256 + 128 * mi : 256 + 128 * mi + 128]
                                p3 = wt[:, k, 512 + 128 * mi : 512 + 128 * mi + 128]
                                st_ = k == 0 and mi == 0
                                sp_ = k == KT - 1 and mi == 1
                                nc.tensor.matmul(
                                    k1t[:, mi, :], p1, a_t[:, k, :],
                                    start=st_, stop=sp_, skip_group_check=True,
                                )
                                nc.tensor.matmul(
                                    k2t[:, mi, :], p2, b_t[:, k, :],
                                    start=st_, stop=sp_, skip_group_check=True,
                                )
                                nc.tensor.matmul(
                                    k3t[:, mi, :], p3, s_t[:, k, :],
                                    start=st_, stop=sp_, skip_group_check=True,
                                )
                        # PE filler while this pass's PSUM drains
                        phase_tile(fill_l, fill_cs, 2 * pass_)
                        phase_tile(fill_l, fill_cs, 2 * pass_ + 1)
                        # ---- pair-wide evacuation + fused phase multiply
                        mg2 = slice(2 * pass_, 2 * pass_ + 2)
                        # re = k1-k2, im = (k3-k1)-k2. Pool cannot touch
                        # PSUM, so ACT stages k1/k3 into SBUF and DVE does
                        # the k2-PSUM-reading subtractions. HW gpsimd runs
                        # these ops far below the model's rate: keep Pool
                        # to at most two ops per pass
                        c1 = upool.tile([128, 2, BC], F32, tag="c1", name="c1", bufs=2)
                        nc.scalar.copy(c1[:], k1t[:])
                        c3 = upool.tile([128, 2, BC], F32, tag="c3", name="c3", bufs=2)
                        nc.scalar.copy(c3[:], k3t[:])
                        re = upool.tile([128, 2, BC], F16, tag="re", name="re", bufs=2)
                        nc.vector.tensor_sub(re[:], c1[:], k2t[:])
                        t2 = upool.tile([128, 2, BC], F32, tag="t2", name="t2", bufs=2)
                        nc.gpsimd.tensor_sub(t2[:], c3[:], c1[:])
                        im = upool.tile([128, 2, BC], F16, tag="im", name="im", bufs=2)
                        nc.vector.tensor_sub(im[:], t2[:], k2t[:])
                        if l < L:
                            stp = csl[:, mg2, 0, :]
                            ctp = csl[:, mg2, 1, :]
                            u1 = upool.tile([128, 2, BC], F16, tag="u1", name="u1", bufs=2)
                            u2 = upool.tile([128, 2, BC], F16, tag="u2", name="u2", bufs=2)
                            u3 = upool.tile([128, 2, BC], F16, tag="u3", name="u3", bufs=2)
                            u4 = upool.tile([128, 2, BC], F16, tag="u4", name="u4", bufs=2)
                            nc.vector.tensor_mul(u1[:], ctp, re[:])
                            nc.vector.tensor_mul(u2[:], stp, im[:])
                            nc.vector.tensor_add(a2_t[:, mg2, :], u1[:], u2[:])
                            nc.gpsimd.tensor_mul(u3[:], ctp, im[:])
                            nc.vector.tensor_mul(u4[:], stp, re[:])
                            nc.vector.tensor_sub(b2_t[:, mg2, :], u3[:], u4[:])
                            nc.vector.tensor_add(
                                s2_t[:, mg2, :], a2_t[:, mg2, :], b2_t[:, mg2, :]
                            )
                        else:
                            # |psi|^2 per element (Zsign reduce after)
                            u1 = upool.tile([128, 2, BC], F16, tag="u1", name="q1", bufs=2)
                            nc.scalar.activation(
                                u1[:], re[:], AF.Square, bias=zbias[:], scale=1.0
                            )
                            u3 = upool.tile([128, 2, BC], F16, tag="u3", name="q2", bufs=2)
                            nc.scalar.activation(
                                u3[:], im[:], AF.Square, bias=zbias[:], scale=1.0
                            )
                            nc.vector.tensor_add(sq_t[:, mg2, :], u1[:], u3[:])
                    if l < L:
                        if dump_debug:
                            nc.sync.dma_start(dbga_d[l], a2_t[:])
                            nc.sync.dma_start(dbgb_d[l], b2_t[:])
                        a_t, b_t, s_t = a2_t, b2_t, s2_t

                # ---- Zsign-weighted partition+tile reduce of |psi|^2
                ez_p = ppool.tile([1, BC], F32, tag="ph", name="ez", bufs=2)
                for mg in range(KT):
                    nc.tensor.matmul(
                        ez_p[:],
                        zs_t[:, mg : mg + 1],
                        sq_t[:, mg, :],
                        start=(mg == 0),
                        stop=(mg == KT - 1),
                        skip_group_check=True,
                    )

                # ---- readout: 2-way softmax == sigmoid of logit gap
                p0 = opool.tile([1, BC], F32, tag="p0", name="p0")
                nc.scalar.activation(
                    p0[:], ez_p[:], AF.Sigmoid, bias=zb1[:], scale=wsc_t[:, :]
                )
                p1 = opool.tile([1, BC], F32, tag="p1", name="p1")
                nc.vector.tensor_scalar(p1[:], p0[:], -1.0, 1.0, ALU.mult, ALU.add)
                nc.sync.dma_start(probs_d[0:1, :], p0[:])
                nc.sync.dma_start(probs_d[1:2, :], p1[:])
                return csn

            cs_pre = None
            for _rep in range(repeat):
                cs_pre = emit_round(debug and _rep == 0, cs_pre)

    nc.finalize()
    _legalize_single_wait(nc)
    return nc


_NC_CACHE = {}


def _get_nc(mm_f32r, debug=False, repeat=1, internal_weights=False):
    key = (bool(mm_f32r), bool(debug), int(repeat), bool(internal_weights))
    if key not in _NC_CACHE:
        _NC_CACHE[key] = _build_nc(
            mm_f32r=key[0], debug=key[1], repeat=key[2], internal_weights=key[3]
        )
    return _NC_CACHE[key]


def _make_in_maps(x, theta, lam, w):
    W, psi1, A = _build_weights(theta, lam)
    at = np.zeros((NQ + 2, L, DIM), np.float32)
    at[:NQ] = A.transpose(1, 0, 2) / (2.0 * np.pi)
    at[NQ, 0] = -np.angle(psi1) / (2.0 * np.pi)
    at[NQ + 1] = 0.25
    pmag = np.abs(psi1)
    shared = {
        "at": np.ascontiguousarray(at).astype(np.float16),
        "psire": np.ascontiguousarray(pmag.reshape(KT, 128).T).astype(
            np.float32
        ),
        "psiim": np.ascontiguousarray(-pmag.reshape(KT, 128).T).astype(
            np.float32
        ),
        "zs": np.ascontiguousarray(_ZSIGN.reshape(KT, 128).T).astype(np.float16),
        "wsc": np.array([[BETA * (float(w[0, 0]) - float(w[0, 1]))]], np.float32),
    }
    for l in range(1, L + 1):
        WT = W[l - 1].T

        def _pack(plane):
            # [1024, 1024] -> [KT, 4pass, 128part, 2mi x 128cols]
            return plane.reshape(KT, 128, 4, 256).transpose(0, 2, 1, 3)

        p1 = _pack(WT.real)
        p2 = _pack(WT.imag)
        shared[f"wall{l}"] = np.ascontiguousarray(
            np.concatenate([p1, p2, p1 + p2], axis=3).transpose(1, 2, 0, 3)
        ).astype(np.float16)
    x = np.asarray(x, np.float32)
    in_maps = []
    for i in range(NC):
        m = dict(shared)
        xt = np.ones((NQ + 2, 2 * BC), np.float32)
        xc = x[BC * i : BC * (i + 1)].T
        xt[:NQ, :BC] = xc
        xt[:NQ, BC:] = xc
        xt[NQ + 1, :BC] = 0.0
        m["xt"] = np.ascontiguousarray(xt).astype(np.float16)
        in_maps.append(m)
    return in_maps


def run(x, theta, lam, w, trace=False, mm_f32r=False, debug=False, repeat=1):
    nc = _get_nc(mm_f32r, debug, repeat)
    in_maps = _make_in_maps(x, theta, lam, w)
    res = run_bass_kernel_spmd(nc, in_maps, list(range(NC)), trace=trace)
    out = np.empty((B, 2), np.float32)
    for i in range(NC):
        out[BC * i : BC * (i + 1)] = res.results[i]["probs"].T
    return out, res


def kernel(x, theta, lam, w):
    out, _ = run(x, theta, lam, w, trace=False, mm_f32r=True)
    return out
